# revision 7
# baseline (speedup 1.0000x reference)
"""Trainium2 Bass kernel for sigmoid-gated multi-head attention.

Reference computation (B=4, F=256, H=8, S=1024):
    qx  = q_input^T          (b, s, f)
    q   = qx @ Wq  -> (b, s, f, h)   [col fi*H + hi]
    k,v = kvx @ Wk / Wv
    attn = sigmoid(sqrt(F) * q.k)    per head
    wv   = attn @ v
    out  = relu(concat_heads(wv) @ Wz)   returned as (b, f, s)

Sharding: 8 cores = 4 batches x 2 query-sequence halves. Each core
computes the full pipeline (all 8 heads) for its (batch, s-half) slice,
including the final ReLU, so per-core outputs are disjoint slices of the
final output and no cross-core reduction is needed.

All on-chip compute keeps the "transposed" layout (feature, sequence):
    QT_h (f, i)  = Wq_h^T @ q_in       KT_h (f, j) = Wk_h^T @ kv_in
    V_hp (j, f2) = kv_in^T @ [Wv_h|Wv_h+1]   (head PAIR -> N=512 matmuls)
    attnT_h (j, i) = sigmoid(16 * KT_h^T_slice . QT_h)
    wvT_h (f, i) = V_slice^T @ attnT_h
    outT (fo, i) += Wz_h^T @ wvT_h     -> relu -> output slice

Startup: the critical first loads are spread across three DMA queues so
compute can start ~3us earlier than a two-queue layout:
    sync ring:   wqk[h0] (wq+wk), qin chunk 0, qin chunk 1
    gpsimd ring: kvin chunk 0, kvin chunk 1 (SWDGE)
    scalar ring: wvz[pair0] + all later per-head weights (the scalar
                 queue starts ~2us late behind the ACT table load)
A short warmup matmul burst bridges the DMA window and releases the PE
HAM clock gate before real matmuls arrive.

Per-head stage order:  attn -> qproj(h+1) -> V-pair (even h) -> wv
(c0/c1 interleaved) -> kt(h+1) -> zproj(h).  This keeps PE work between
every producer/consumer pair (sigmoid chain, PSUM->SBUF copies) so the
matmul stream never waits.
"""

import os
import sys

sys.path.insert(0, "/opt/trn_rl_repo")

import numpy as np

B, F, H, S = 4, 256, 8, 1024
HALF = S // 2  # query columns per core
NCORES = 8
P = 128  # partitions

_cache = {}


def _build():
    import concourse.mybir as mybir
    import concourse.tile as tile
    from concourse import bacc

    dt = mybir.dt
    f32 = dt.float32
    mdt = dt.float16
    AF = mybir.ActivationFunctionType

    nc = bacc.Bacc(None, target_bir_lowering=False)

    # all partition-major: [P, ...] with per-partition lines contiguous
    qin_d = nc.dram_tensor("qin", [P, 2, HALF], mdt, kind="ExternalInput")
    kvin_d = nc.dram_tensor("kvin", [P, 2, S], mdt, kind="ExternalInput")
    # per head: [wq|wk][f_in chunk][f_out]
    wqk_d = nc.dram_tensor("wqk", [H, P, 2, 2, F], mdt, kind="ExternalInput")
    # per head-pair: cols 0:512 = [Wv_h|Wv_h+1], 512:768 = Wz_h, 768:1024 = Wz_h+1
    wvz_d = nc.dram_tensor("wvz", [H // 2, P, 2, 1024], mdt, kind="ExternalInput")
    out_d = nc.dram_tensor("out", [P, 2, HALF], mdt, kind="ExternalOutput")

    with tile.TileContext(nc) as tc:
        with (
            tc.tile_pool(name="io", bufs=1) as io_pool,
            tc.tile_pool(name="wts", bufs=2) as w_pool,
            tc.tile_pool(name="qkv", bufs=2) as qkv_pool,
            tc.tile_pool(name="attn", bufs=2) as attn_pool,
            tc.tile_pool(name="ps", bufs=6, space="PSUM") as ps_pool,
            tc.tile_pool(name="ops", bufs=1, space="PSUM") as out_ps_pool,
        ):
            # ---- input DMAs: critical loads first, spread across queues
            wqk = {0: w_pool.tile([P, 2, 2, F], mdt, tag="wqk", name="wqk0")}
            nc.sync.dma_start(wqk[0][:], wqk_d[0])
            qin = io_pool.tile([P, 2, HALF], mdt, tag="qin")
            nc.sync.dma_start(qin[:, 0], qin_d[:, 0])
            nc.sync.dma_start(qin[:, 1], qin_d[:, 1])
            kvin = [
                io_pool.tile([P, S], mdt, tag=f"kvin{c}", name=f"kvin{c}")
                for c in range(2)
            ]
            nc.gpsimd.dma_start(kvin[0][:], kvin_d[:, 0])
            nc.gpsimd.dma_start(kvin[1][:], kvin_d[:, 1])
            wvz = {0: w_pool.tile([P, 2, 1024], mdt, tag="wvz", name="wvz0")}
            nc.scalar.dma_start(wvz[0][:], wvz_d[0])

            # ---- PE pre-warm: dummy matmuls bridge the input-DMA window and
            # release the HAM clock gate before real matmuls arrive.
            nwarm = int(os.environ.get("ATTN_NWARM", "6"))
            if nwarm:
                warm = io_pool.tile([P, HALF], dt.bfloat16, tag="warm")
                nc.gpsimd.memset(warm[:], 0.0)
                wps = [
                    ps_pool.tile([P, HALF], f32, tag="ps", name=f"wps{i}")
                    for i in range(2)
                ]
                for i in range(nwarm):
                    nc.tensor.matmul(
                        wps[i % 2][:], warm[:, :P], warm[:], start=True, stop=True
                    )

            # persistent accumulator for the output projection: 2 banks
            out_ps = out_ps_pool.tile([P, 2, HALF], f32, tag="out_ps")

            def q_proj(h):
                """QT_h (f 2x128, i 512) = Wq_h^T @ qin; returns qt tile."""
                qt = qkv_pool.tile([P, 2, HALF], mdt, tag="qt", name=f"qt{h}")
                for t in range(2):
                    ps = ps_pool.tile([P, HALF], f32, tag="ps", name=f"psq{h}{t}")
                    for c in range(2):
                        nc.tensor.matmul(
                            ps[:],
                            wqk[h][:, 0, c, P * t : P * (t + 1)],
                            qin[:, c, :],
                            start=(c == 0),
                            stop=(c == 1),
                        )
                    if t == 0:
                        nc.vector.tensor_copy(qt[:, t, :], ps[:])
                    else:
                        nc.scalar.activation(qt[:, t, :], ps[:], AF.Copy)
                return qt

            def kt_mm(h):
                """KT_h (f 2x128, j 1024) = Wk_h^T @ kvin; returns kt tile."""
                kt = qkv_pool.tile([P, 2, S], mdt, tag="kt", name=f"kt{h}")
                for t in range(2):
                    for n in range(2):
                        ps = ps_pool.tile([P, HALF], f32, tag="ps")
                        for c in range(2):
                            nc.tensor.matmul(
                                ps[:],
                                wqk[h][:, 1, c, P * t : P * (t + 1)],
                                kvin[c][:, HALF * n : HALF * (n + 1)],
                                start=(c == 0),
                                stop=(c == 1),
                            )
                        if (t + n) % 2 == 0:
                            nc.vector.tensor_copy(
                                kt[:, t, HALF * n : HALF * (n + 1)], ps[:]
                            )
                        else:
                            nc.scalar.activation(
                                kt[:, t, HALF * n : HALF * (n + 1)], ps[:], AF.Copy
                            )
                return kt

            qt_cur = q_proj(0)
            kt_cur = kt_mm(0)
            v_cur = None

            for h in range(H):
                # prefetch next head's / pair's weights on the scalar ring
                if h + 1 < H:
                    wqk[h + 1] = w_pool.tile(
                        [P, 2, 2, F], mdt, tag="wqk", name=f"wqk{h + 1}"
                    )
                    nc.scalar.dma_start(wqk[h + 1][:], wqk_d[h + 1])
                if h % 2 == 0 and h + 2 < H:
                    hp = (h + 2) // 2
                    wvz[hp] = w_pool.tile(
                        [P, 2, 1024], mdt, tag="wvz", name=f"wvz{hp}"
                    )
                    nc.scalar.dma_start(wvz[hp][:], wvz_d[hp])

                # attnT_h (j 8x128, i 512) = sigmoid(16 * KT_slice^T @ QT)
                atn = attn_pool.tile([P, 8, HALF], mdt, tag="atn")
                for jb in range(8):
                    ps = ps_pool.tile([P, HALF], f32, tag="ps")
                    for c in range(2):
                        nc.tensor.matmul(
                            ps[:],
                            kt_cur[:, c, P * jb : P * (jb + 1)],
                            qt_cur[:, c, :],
                            start=(c == 0),
                            stop=(c == 1),
                        )
                    nc.scalar.activation(atn[:, jb, :], ps[:], AF.Sigmoid, scale=16.0)

                # software-pipeline: next head's Q projection keeps the PE fed
                # while the sigmoid chain drains.
                if h + 1 < H:
                    qt_next = q_proj(h + 1)

                # V for the head PAIR (j 8x128, f 2x256) = kvin^T @ [Wv_h|Wv_h+1]
                if h % 2 == 0:
                    v_cur = qkv_pool.tile([P, 8, 2 * F], mdt, tag="v")
                    for jb in range(8):
                        ps = ps_pool.tile([P, HALF], f32, tag="ps")
                        for c in range(2):
                            nc.tensor.matmul(
                                ps[:],
                                kvin[c][:, P * jb : P * (jb + 1)],
                                wvz[h // 2][:, c, 0:512],
                                start=(c == 0),
                                stop=(c == 1),
                            )
                        nc.vector.tensor_copy(v_cur[:, jb, :], ps[:])

                voff = (h % 2) * F  # this head's columns inside v_cur

                def wz_sl(c, t):
                    base = 512 + 256 * (h % 2)
                    return wvz[h // 2][:, c, base + P * t : base + P * (t + 1)]

                if h < H - 1:
                    # wvT_h (f 2x128, i 512): c0/c1 accumulation chains
                    # interleaved so the chain ends land well after the last
                    # sigmoid and neither chain stalls.
                    wvt = qkv_pool.tile([P, 2, HALF], mdt, tag="wvt")
                    psc = [
                        ps_pool.tile([P, HALF], f32, tag="ps", name=f"pswv{h}{c}")
                        for c in range(2)
                    ]
                    for jb in range(8):
                        for c in range(2):
                            nc.tensor.matmul(
                                psc[c][:],
                                v_cur[:, jb, voff + P * c : voff + P * (c + 1)],
                                atn[:, jb, :],
                                start=(jb == 0),
                                stop=(jb == 7),
                            )
                    nc.vector.tensor_copy(wvt[:, 0, :], psc[0][:])
                    nc.scalar.activation(wvt[:, 1, :], psc[1][:], AF.Copy)

                    # next head's K projection pads the PE stream across the
                    # wvt-copy latency before the output projection.
                    kt_next = kt_mm(h + 1)

                    # outT (fo 2x128, i 512) += Wz_h[c]^T @ wvT[c]
                    for c in range(2):
                        for t in range(2):
                            nc.tensor.matmul(
                                out_ps[:, t, :],
                                wz_sl(c, t),
                                wvt[:, c, :],
                                start=(h == 0 and c == 0),
                                stop=False,
                            )
                    kt_cur = kt_next
                    qt_cur = qt_next
                else:
                    # last head: split by i-half so copy/projection of half 0
                    # overlap half 1's accumulation, shortening the tail.
                    out_sb = io_pool.tile([P, 2, HALF], mdt, tag="out_sb")
                    wvt = qkv_pool.tile([P, 2, HALF], mdt, tag="wvt")
                    for ih in range(2):
                        sl = slice(F * ih, F * (ih + 1))
                        pst = [
                            ps_pool.tile([P, HALF], f32, tag="ps", name=f"pst{ih}{c}")
                            for c in range(2)
                        ]
                        for jb in range(8):
                            for c in range(2):
                                nc.tensor.matmul(
                                    pst[c][:, :F],
                                    v_cur[:, jb, voff + P * c : voff + P * (c + 1)],
                                    atn[:, jb, sl],
                                    start=(jb == 0),
                                    stop=(jb == 7),
                                )
                        nc.vector.tensor_copy(wvt[:, 0, sl], pst[0][:, :F])
                        nc.scalar.activation(wvt[:, 1, sl], pst[1][:, :F], AF.Copy)
                        for t in range(2):
                            for c in range(2):
                                nc.tensor.matmul(
                                    out_ps[:, t, sl],
                                    wz_sl(c, t),
                                    wvt[:, c, sl],
                                    start=False,
                                    stop=(c == 1 and ih == 1),
                                )
                    # drain: relus on both engines, output DMA on both rings
                    nc.vector.tensor_relu(out_sb[:, 0, :], out_ps[:, 0, :])
                    nc.sync.dma_start(out_d[:, 0], out_sb[:, 0, :])
                    nc.scalar.activation(out_sb[:, 1, :], out_ps[:, 1, :], AF.Relu)
                    nc.scalar.dma_start(out_d[:, 1], out_sb[:, 1, :])

    nc.compile()
    return nc


def _get_nc():
    if "nc" not in _cache:
        _cache["nc"] = _build()
    return _cache["nc"]


def _make_in_maps(inputs):
    ndt = np.float16
    q_input = np.asarray(inputs["q_input"], dtype=np.float32)
    kv_input = np.asarray(inputs["kv_input"], dtype=np.float32)

    # Wq/Wk [f_in, fo*H] (col fi*H+hi) -> [h, p, c, fo]
    def cols_by_head(W):
        return np.asarray(W, dtype=np.float32).reshape(2, P, F, H).transpose(3, 1, 0, 2)

    WQ = cols_by_head(inputs["Wq"])  # [H, P, 2, F]
    WK = cols_by_head(inputs["Wk"])
    WQK = np.ascontiguousarray(
        np.stack([WQ, WK], axis=2), dtype=ndt
    )  # [H, P, 2(q|k), 2c, F]
    # Wv -> pair layout [4, P, 2c, 512]
    WV = cols_by_head(inputs["Wv"])  # [H, P, 2, F]
    WVP = (
        WV.reshape(4, 2, P, 2, F).transpose(0, 2, 3, 1, 4).reshape(4, P, 2, 2 * F)
    )
    # Wz [f*H, fo] (row fi*H+hi) -> [h, p, c, fo]
    WZ = (
        np.asarray(inputs["Wz"], dtype=np.float32)
        .reshape(2, P, H, F)
        .transpose(2, 1, 0, 3)
    )  # [H, P, 2, F]
    WZP = WZ.reshape(4, 2, P, 2, F).transpose(0, 2, 3, 1, 4).reshape(4, P, 2, 2 * F)
    WVZ = np.ascontiguousarray(
        np.concatenate([WVP, WZP], axis=3), dtype=ndt
    )  # [4, P, 2, 1024]

    in_maps = []
    for core in range(NCORES):
        b, half = divmod(core, 2)
        qb = q_input[b].reshape(2, P, S)
        qin = np.ascontiguousarray(
            qb[:, :, half * HALF : (half + 1) * HALF].transpose(1, 0, 2), dtype=ndt
        )
        kvin = np.ascontiguousarray(
            kv_input[b].reshape(2, P, S).transpose(1, 0, 2), dtype=ndt
        )
        in_maps.append({"qin": qin, "kvin": kvin, "wqk": WQK, "wvz": WVZ})
    return in_maps


def kernel(q_input, kv_input, Wq, Wk, Wv, Wz, **kw):
    from concourse.bass_utils import run_bass_kernel_spmd

    nc = _get_nc()
    in_maps = _make_in_maps(
        {
            "q_input": q_input,
            "kv_input": kv_input,
            "Wq": Wq,
            "Wk": Wk,
            "Wv": Wv,
            "Wz": Wz,
        }
    )

    res = run_bass_kernel_spmd(nc, in_maps, core_ids=list(range(NCORES)))

    out = np.empty((B, F, S), dtype=np.float32)
    for core in range(NCORES):
        b, half = divmod(core, 2)
        o = np.asarray(res.results[core]["out"], dtype=np.float32)  # (P, 2, HALF)
        out[b, :, half * HALF : (half + 1) * HALF] = o.transpose(1, 0, 2).reshape(
            F, HALF
        )
    return out


# revision 10
# speedup vs baseline: 1.0092x; 1.0092x over previous
"""Trainium2 Bass kernel for sigmoid-gated multi-head attention.

Reference computation (B=4, F=256, H=8, S=1024):
    qx  = q_input^T          (b, s, f)
    q   = qx @ Wq  -> (b, s, f, h)   [col fi*H + hi]
    k,v = kvx @ Wk / Wv
    attn = sigmoid(sqrt(F) * q.k)    per head
    wv   = attn @ v
    out  = relu(concat_heads(wv) @ Wz)   returned as (b, f, s)

Sharding: 8 cores = 4 batches x 2 query-sequence halves. Each core
computes the full pipeline (all 8 heads) for its (batch, s-half) slice,
including the final ReLU, so per-core outputs are disjoint slices of the
final output and no cross-core reduction is needed.

All on-chip compute keeps the "transposed" layout (feature, sequence):
    QT_h (f, i)  = Wq_h^T @ q_in       KT_h (f, j) = Wk_h^T @ kv_in
    V_hp (j, f2) = kv_in^T @ [Wv_h|Wv_h+1]   (head PAIR -> N=512 matmuls)
    attnT_h (j, i) = sigmoid(16 * KT_h^T_slice . QT_h)
    wvT_h (f, i) = V_slice^T @ attnT_h
    outT (fo, i) += Wz_h^T @ wvT_h     -> relu -> output slice

Startup: the critical first loads are spread across three DMA queues so
compute can start ~3us earlier than a two-queue layout:
    sync ring:   wqk[h0] (wq+wk), qin chunk 0, qin chunk 1
    gpsimd ring: kvin chunk 0, kvin chunk 1 (SWDGE)
    scalar ring: wvz[pair0] + all later per-head weights (the scalar
                 queue starts ~2us late behind the ACT table load)
A short warmup matmul burst bridges the DMA window and releases the PE
HAM clock gate before real matmuls arrive.

Per-head stage order:  attn -> qproj(h+1) -> V-pair (even h) -> wv
(c0/c1 interleaved) -> kt(h+1) -> zproj(h).  This keeps PE work between
every producer/consumer pair (sigmoid chain, PSUM->SBUF copies) so the
matmul stream never waits.
"""

import os
import sys

sys.path.insert(0, "/opt/trn_rl_repo")

import numpy as np

B, F, H, S = 4, 256, 8, 1024
HALF = S // 2  # query columns per core
NCORES = 8
P = 128  # partitions

_cache = {}


def _build():
    import concourse.mybir as mybir
    import concourse.tile as tile
    from concourse import bacc

    dt = mybir.dt
    f32 = dt.float32
    mdt = dt.float16
    AF = mybir.ActivationFunctionType

    nc = bacc.Bacc(None, target_bir_lowering=False)

    # all partition-major: [P, ...] with per-partition lines contiguous
    qin_d = nc.dram_tensor("qin", [P, 2, HALF], mdt, kind="ExternalInput")
    kvin_d = nc.dram_tensor("kvin", [P, 2, S], mdt, kind="ExternalInput")
    # per head: [wq|wk][f_in chunk][f_out]
    wqk_d = nc.dram_tensor("wqk", [H, P, 2, 2, F], mdt, kind="ExternalInput")
    # per head-pair: cols 0:512 = [Wv_h|Wv_h+1], 512:768 = Wz_h, 768:1024 = Wz_h+1
    wvz_d = nc.dram_tensor("wvz", [H // 2, P, 2, 1024], mdt, kind="ExternalInput")
    out_d = nc.dram_tensor("out", [P, 2, HALF], mdt, kind="ExternalOutput")

    with tile.TileContext(nc) as tc:
        with (
            tc.tile_pool(name="io", bufs=1) as io_pool,
            tc.tile_pool(name="wts", bufs=2) as w_pool,
            tc.tile_pool(name="qkv", bufs=2) as qkv_pool,
            tc.tile_pool(name="attn", bufs=2) as attn_pool,
            tc.tile_pool(name="ps", bufs=6, space="PSUM") as ps_pool,
            tc.tile_pool(name="ops", bufs=1, space="PSUM") as out_ps_pool,
        ):
            # ---- input DMAs: critical loads first, HWDGE rings only.
            # sync ring (live at ~+0.6us): wq0, qin, wk0, kvin chunk 1.
            # scalar ring (live at ~+3us behind the ACT table load):
            # kvin chunk 0, wvz pair 0, then the per-head weight stream.
            wqk = {0: w_pool.tile([P, 2, 2, F], mdt, tag="wqk", name="wqk0")}
            nc.sync.dma_start(wqk[0][:, 0], wqk_d[0][:, 0])
            qin = io_pool.tile([P, 2, HALF], mdt, tag="qin")
            nc.sync.dma_start(qin[:, 0], qin_d[:, 0])
            nc.sync.dma_start(qin[:, 1], qin_d[:, 1])
            nc.sync.dma_start(wqk[0][:, 1], wqk_d[0][:, 1])
            kvin = [
                io_pool.tile([P, S], mdt, tag=f"kvin{c}", name=f"kvin{c}")
                for c in range(2)
            ]
            nc.scalar.dma_start(kvin[0][:], kvin_d[:, 0])
            nc.sync.dma_start(kvin[1][:], kvin_d[:, 1])
            wvz = {0: w_pool.tile([P, 2, 1024], mdt, tag="wvz", name="wvz0")}
            nc.scalar.dma_start(wvz[0][:], wvz_d[0])

            # ---- PE pre-warm: dummy matmuls bridge the input-DMA window and
            # release the HAM clock gate before real matmuls arrive.
            nwarm = int(os.environ.get("ATTN_NWARM", "3"))
            if nwarm:
                warm = io_pool.tile([P, HALF], dt.bfloat16, tag="warm")
                nc.gpsimd.memset(warm[:], 0.0)
                wps = [
                    ps_pool.tile([P, HALF], f32, tag="ps", name=f"wps{i}")
                    for i in range(2)
                ]
                for i in range(nwarm):
                    nc.tensor.matmul(
                        wps[i % 2][:], warm[:, :P], warm[:], start=True, stop=True
                    )

            # persistent accumulator for the output projection: 2 banks
            out_ps = out_ps_pool.tile([P, 2, HALF], f32, tag="out_ps")

            def q_proj(h):
                """QT_h (f 2x128, i 512) = Wq_h^T @ qin; returns qt tile."""
                qt = qkv_pool.tile([P, 2, HALF], mdt, tag="qt", name=f"qt{h}")
                for t in range(2):
                    ps = ps_pool.tile([P, HALF], f32, tag="ps", name=f"psq{h}{t}")
                    for c in range(2):
                        nc.tensor.matmul(
                            ps[:],
                            wqk[h][:, 0, c, P * t : P * (t + 1)],
                            qin[:, c, :],
                            start=(c == 0),
                            stop=(c == 1),
                        )
                    if t == 0:
                        nc.vector.tensor_copy(qt[:, t, :], ps[:])
                    else:
                        nc.scalar.activation(qt[:, t, :], ps[:], AF.Copy)
                return qt

            def kt_mm(h):
                """KT_h (f 2x128, j 1024) = Wk_h^T @ kvin; returns kt tile."""
                kt = qkv_pool.tile([P, 2, S], mdt, tag="kt", name=f"kt{h}")
                for t in range(2):
                    for n in range(2):
                        ps = ps_pool.tile([P, HALF], f32, tag="ps")
                        for c in range(2):
                            nc.tensor.matmul(
                                ps[:],
                                wqk[h][:, 1, c, P * t : P * (t + 1)],
                                kvin[c][:, HALF * n : HALF * (n + 1)],
                                start=(c == 0),
                                stop=(c == 1),
                            )
                        if (t + n) % 2 == 0:
                            nc.vector.tensor_copy(
                                kt[:, t, HALF * n : HALF * (n + 1)], ps[:]
                            )
                        else:
                            nc.scalar.activation(
                                kt[:, t, HALF * n : HALF * (n + 1)], ps[:], AF.Copy
                            )
                return kt

            qt_cur = q_proj(0)
            kt_cur = kt_mm(0)
            v_cur = None

            for h in range(H):
                # prefetch next head's / pair's weights on the scalar ring
                if h + 1 < H:
                    wqk[h + 1] = w_pool.tile(
                        [P, 2, 2, F], mdt, tag="wqk", name=f"wqk{h + 1}"
                    )
                    nc.scalar.dma_start(wqk[h + 1][:], wqk_d[h + 1])
                if h % 2 == 0 and h + 2 < H:
                    hp = (h + 2) // 2
                    wvz[hp] = w_pool.tile(
                        [P, 2, 1024], mdt, tag="wvz", name=f"wvz{hp}"
                    )
                    # late bulk weights ride the otherwise-idle SWDGE queue
                    nc.gpsimd.dma_start(wvz[hp][:], wvz_d[hp])

                # attnT_h (j 8x128, i 512) = sigmoid(16 * KT_slice^T @ QT)
                atn = attn_pool.tile([P, 8, HALF], mdt, tag="atn")
                for jb in range(8):
                    ps = ps_pool.tile([P, HALF], f32, tag="ps")
                    for c in range(2):
                        nc.tensor.matmul(
                            ps[:],
                            kt_cur[:, c, P * jb : P * (jb + 1)],
                            qt_cur[:, c, :],
                            start=(c == 0),
                            stop=(c == 1),
                        )
                    nc.scalar.activation(atn[:, jb, :], ps[:], AF.Sigmoid, scale=16.0)

                # software-pipeline: next head's Q projection keeps the PE fed
                # while the sigmoid chain drains.
                if h + 1 < H:
                    qt_next = q_proj(h + 1)

                # V for the head PAIR (j 8x128, f 2x256) = kvin^T @ [Wv_h|Wv_h+1]
                if h % 2 == 0:
                    v_cur = qkv_pool.tile([P, 8, 2 * F], mdt, tag="v")
                    for jb in range(8):
                        ps = ps_pool.tile([P, HALF], f32, tag="ps")
                        for c in range(2):
                            nc.tensor.matmul(
                                ps[:],
                                kvin[c][:, P * jb : P * (jb + 1)],
                                wvz[h // 2][:, c, 0:512],
                                start=(c == 0),
                                stop=(c == 1),
                            )
                        nc.vector.tensor_copy(v_cur[:, jb, :], ps[:])

                voff = (h % 2) * F  # this head's columns inside v_cur

                def wz_sl(c, t):
                    base = 512 + 256 * (h % 2)
                    return wvz[h // 2][:, c, base + P * t : base + P * (t + 1)]

                # wvT_h (f 2x128, i 512): c0/c1 accumulation chains
                # interleaved so the chain ends land well after the last
                # sigmoid and neither chain stalls.
                wvt = qkv_pool.tile([P, 2, HALF], mdt, tag="wvt")
                psc = [
                    ps_pool.tile([P, HALF], f32, tag="ps", name=f"pswv{h}{c}")
                    for c in range(2)
                ]
                for jb in range(8):
                    for c in range(2):
                        nc.tensor.matmul(
                            psc[c][:],
                            v_cur[:, jb, voff + P * c : voff + P * (c + 1)],
                            atn[:, jb, :],
                            start=(jb == 0),
                            stop=(jb == 7),
                        )
                nc.vector.tensor_copy(wvt[:, 0, :], psc[0][:])
                nc.scalar.activation(wvt[:, 1, :], psc[1][:], AF.Copy)

                if h < H - 1:
                    # next head's K projection pads the PE stream across the
                    # wvt-copy latency before the output projection.
                    kt_next = kt_mm(h + 1)

                    # outT (fo 2x128, i 512) += Wz_h[c]^T @ wvT[c]
                    for c in range(2):
                        for t in range(2):
                            nc.tensor.matmul(
                                out_ps[:, t, :],
                                wz_sl(c, t),
                                wvt[:, c, :],
                                start=(h == 0 and c == 0),
                                stop=False,
                            )
                    kt_cur = kt_next
                    qt_cur = qt_next
                else:
                    # last head: c-major projection order completes PSUM bank
                    # t as early as possible; relu+DMA per t on both engine
                    # pairs / rings so the drain overlaps.
                    out_sb = io_pool.tile([P, 2, HALF], mdt, tag="out_sb")
                    for c in range(2):
                        for t in range(2):
                            nc.tensor.matmul(
                                out_ps[:, t, :],
                                wz_sl(c, t),
                                wvt[:, c, :],
                                start=False,
                                stop=(c == 1),
                            )
                    nc.vector.tensor_relu(out_sb[:, 0, :], out_ps[:, 0, :])
                    nc.sync.dma_start(out_d[:, 0], out_sb[:, 0, :])
                    nc.scalar.activation(out_sb[:, 1, :], out_ps[:, 1, :], AF.Relu)
                    nc.scalar.dma_start(out_d[:, 1], out_sb[:, 1, :])

    nc.compile()
    return nc


def _get_nc():
    if "nc" not in _cache:
        _cache["nc"] = _build()
    return _cache["nc"]


def _make_in_maps(inputs):
    ndt = np.float16
    q_input = np.asarray(inputs["q_input"], dtype=np.float32)
    kv_input = np.asarray(inputs["kv_input"], dtype=np.float32)

    # Wq/Wk [f_in, fo*H] (col fi*H+hi) -> [h, p, c, fo]
    def cols_by_head(W):
        return np.asarray(W, dtype=np.float32).reshape(2, P, F, H).transpose(3, 1, 0, 2)

    WQ = cols_by_head(inputs["Wq"])  # [H, P, 2, F]
    WK = cols_by_head(inputs["Wk"])
    WQK = np.ascontiguousarray(
        np.stack([WQ, WK], axis=2), dtype=ndt
    )  # [H, P, 2(q|k), 2c, F]
    # Wv -> pair layout [4, P, 2c, 512]
    WV = cols_by_head(inputs["Wv"])  # [H, P, 2, F]
    WVP = (
        WV.reshape(4, 2, P, 2, F).transpose(0, 2, 3, 1, 4).reshape(4, P, 2, 2 * F)
    )
    # Wz [f*H, fo] (row fi*H+hi) -> [h, p, c, fo]
    WZ = (
        np.asarray(inputs["Wz"], dtype=np.float32)
        .reshape(2, P, H, F)
        .transpose(2, 1, 0, 3)
    )  # [H, P, 2, F]
    WZP = WZ.reshape(4, 2, P, 2, F).transpose(0, 2, 3, 1, 4).reshape(4, P, 2, 2 * F)
    WVZ = np.ascontiguousarray(
        np.concatenate([WVP, WZP], axis=3), dtype=ndt
    )  # [4, P, 2, 1024]

    in_maps = []
    for core in range(NCORES):
        b, half = divmod(core, 2)
        qb = q_input[b].reshape(2, P, S)
        qin = np.ascontiguousarray(
            qb[:, :, half * HALF : (half + 1) * HALF].transpose(1, 0, 2), dtype=ndt
        )
        kvin = np.ascontiguousarray(
            kv_input[b].reshape(2, P, S).transpose(1, 0, 2), dtype=ndt
        )
        in_maps.append({"qin": qin, "kvin": kvin, "wqk": WQK, "wvz": WVZ})
    return in_maps


def kernel(q_input, kv_input, Wq, Wk, Wv, Wz, **kw):
    from concourse.bass_utils import run_bass_kernel_spmd

    nc = _get_nc()
    in_maps = _make_in_maps(
        {
            "q_input": q_input,
            "kv_input": kv_input,
            "Wq": Wq,
            "Wk": Wk,
            "Wv": Wv,
            "Wz": Wz,
        }
    )

    res = run_bass_kernel_spmd(nc, in_maps, core_ids=list(range(NCORES)))

    out = np.empty((B, F, S), dtype=np.float32)
    for core in range(NCORES):
        b, half = divmod(core, 2)
        o = np.asarray(res.results[core]["out"], dtype=np.float32)  # (P, 2, HALF)
        out[b, :, half * HALF : (half + 1) * HALF] = o.transpose(1, 0, 2).reshape(
            F, HALF
        )
    return out


# revision 12
# speedup vs baseline: 1.0132x; 1.0039x over previous
"""Trainium2 Bass kernel for sigmoid-gated multi-head attention.

Reference computation (B=4, F=256, H=8, S=1024):
    qx  = q_input^T          (b, s, f)
    q   = qx @ Wq  -> (b, s, f, h)   [col fi*H + hi]
    k,v = kvx @ Wk / Wv
    attn = sigmoid(sqrt(F) * q.k)    per head
    wv   = attn @ v
    out  = relu(concat_heads(wv) @ Wz)   returned as (b, f, s)

Sharding: 8 cores = 4 batches x 2 query-sequence halves. Each core
computes the full pipeline (all 8 heads) for its (batch, s-half) slice,
including the final ReLU, so per-core outputs are disjoint slices of the
final output and no cross-core reduction is needed.

All on-chip compute keeps the "transposed" layout (feature, sequence):
    QT_h (f, i)  = Wq_h^T @ q_in       KT_h (f, j) = Wk_h^T @ kv_in
    V_hp (j, f2) = kv_in^T @ [Wv_h|Wv_h+1]   (head PAIR -> N=512 matmuls)
    attnT_h (j, i) = sigmoid(16 * KT_h^T_slice . QT_h)
    wvT_h (f, i) = V_slice^T @ attnT_h
    outT (fo, i) += Wz_h^T @ wvT_h     -> relu -> output slice

Startup: the critical first loads are spread across three DMA queues so
compute can start ~3us earlier than a two-queue layout:
    sync ring:   wqk[h0] (wq+wk), qin chunk 0, qin chunk 1
    gpsimd ring: kvin chunk 0, kvin chunk 1 (SWDGE)
    scalar ring: wvz[pair0] + all later per-head weights (the scalar
                 queue starts ~2us late behind the ACT table load)
A short warmup matmul burst bridges the DMA window and releases the PE
HAM clock gate before real matmuls arrive.

Per-head stage order:  attn -> qproj(h+1) -> V-pair (even h) -> wv
(c0/c1 interleaved) -> kt(h+1) -> zproj(h).  This keeps PE work between
every producer/consumer pair (sigmoid chain, PSUM->SBUF copies) so the
matmul stream never waits.
"""

import os
import sys

sys.path.insert(0, "/opt/trn_rl_repo")

import numpy as np

B, F, H, S = 4, 256, 8, 1024
HALF = S // 2  # query columns per core
NCORES = 8
P = 128  # partitions

_cache = {}


def _build():
    import concourse.mybir as mybir
    import concourse.tile as tile
    from concourse import bacc

    dt = mybir.dt
    f32 = dt.float32
    mdt = dt.float16
    AF = mybir.ActivationFunctionType

    nc = bacc.Bacc(None, target_bir_lowering=False)

    # all partition-major: [P, ...] with per-partition lines contiguous
    qin_d = nc.dram_tensor("qin", [P, 2, HALF], mdt, kind="ExternalInput")
    kvin_d = nc.dram_tensor("kvin", [P, 2, S], mdt, kind="ExternalInput")
    # per head: [wq|wk][f_in chunk][f_out]
    wqk_d = nc.dram_tensor("wqk", [H, P, 2, 2, F], mdt, kind="ExternalInput")
    # per head-pair: cols 0:512 = [Wv_h|Wv_h+1], 512:768 = Wz_h, 768:1024 = Wz_h+1
    wvz_d = nc.dram_tensor("wvz", [H // 2, P, 2, 1024], mdt, kind="ExternalInput")
    out_d = nc.dram_tensor("out", [P, 2, HALF], mdt, kind="ExternalOutput")

    with tile.TileContext(nc) as tc:
        with (
            tc.tile_pool(name="io", bufs=1) as io_pool,
            tc.tile_pool(name="wts", bufs=2) as w_pool,
            tc.tile_pool(name="qkv", bufs=2) as qkv_pool,
            tc.tile_pool(name="attn", bufs=2) as attn_pool,
            tc.tile_pool(name="ps", bufs=6, space="PSUM") as ps_pool,
            tc.tile_pool(name="ops", bufs=1, space="PSUM") as out_ps_pool,
        ):
            # ---- input DMAs: critical loads first, HWDGE rings only.
            # sync ring (live at ~+0.6us): wq0, qin, wk0, kvin chunk 1.
            # scalar ring (live at ~+3us behind the ACT table load):
            # kvin chunk 0, wvz pair 0, then the per-head weight stream.
            wqk = {0: w_pool.tile([P, 2, 2, F], mdt, tag="wqk", name="wqk0")}
            nc.sync.dma_start(wqk[0][:, 0], wqk_d[0][:, 0])
            qin = io_pool.tile([P, 2, HALF], mdt, tag="qin")
            nc.sync.dma_start(qin[:, 0], qin_d[:, 0])
            nc.sync.dma_start(qin[:, 1], qin_d[:, 1])
            nc.sync.dma_start(wqk[0][:, 1], wqk_d[0][:, 1])
            kvin = [
                io_pool.tile([P, S], mdt, tag=f"kvin{c}", name=f"kvin{c}")
                for c in range(2)
            ]
            nc.scalar.dma_start(kvin[0][:], kvin_d[:, 0])
            nc.sync.dma_start(kvin[1][:], kvin_d[:, 1])
            wvz = {0: w_pool.tile([P, 2, 1024], mdt, tag="wvz", name="wvz0")}
            nc.scalar.dma_start(wvz[0][:], wvz_d[0])

            # ---- PE pre-warm: dummy matmuls bridge the input-DMA window and
            # release the HAM clock gate before real matmuls arrive.
            nwarm = int(os.environ.get("ATTN_NWARM", "4"))
            if nwarm:
                warm = io_pool.tile([P, HALF], dt.bfloat16, tag="warm")
                nc.gpsimd.memset(warm[:], 0.0)
                wps = [
                    ps_pool.tile([P, HALF], f32, tag="ps", name=f"wps{i}")
                    for i in range(2)
                ]
                for i in range(nwarm):
                    nc.tensor.matmul(
                        wps[i % 2][:], warm[:, :P], warm[:], start=True, stop=True
                    )

            # persistent accumulator for the output projection: 2 banks
            out_ps = out_ps_pool.tile([P, 2, HALF], f32, tag="out_ps")

            def q_proj(h):
                """QT_h (f 2x128, i 512) = Wq_h^T @ qin; returns qt tile."""
                qt = qkv_pool.tile([P, 2, HALF], mdt, tag="qt", name=f"qt{h}")
                for t in range(2):
                    ps = ps_pool.tile([P, HALF], f32, tag="ps", name=f"psq{h}{t}")
                    for c in range(2):
                        nc.tensor.matmul(
                            ps[:],
                            wqk[h][:, 0, c, P * t : P * (t + 1)],
                            qin[:, c, :],
                            start=(c == 0),
                            stop=(c == 1),
                        )
                    if t == 0:
                        nc.vector.tensor_copy(qt[:, t, :], ps[:])
                    else:
                        nc.scalar.activation(qt[:, t, :], ps[:], AF.Copy)
                return qt

            def kt_mm(h):
                """KT_h (f 2x128, j 1024) = Wk_h^T @ kvin; returns kt tile."""
                kt = qkv_pool.tile([P, 2, S], mdt, tag="kt", name=f"kt{h}")
                for t in range(2):
                    for n in range(2):
                        ps = ps_pool.tile([P, HALF], f32, tag="ps")
                        for c in range(2):
                            nc.tensor.matmul(
                                ps[:],
                                wqk[h][:, 1, c, P * t : P * (t + 1)],
                                kvin[c][:, HALF * n : HALF * (n + 1)],
                                start=(c == 0),
                                stop=(c == 1),
                            )
                        if (t + n) % 2 == 0:
                            nc.vector.tensor_copy(
                                kt[:, t, HALF * n : HALF * (n + 1)], ps[:]
                            )
                        else:
                            nc.scalar.activation(
                                kt[:, t, HALF * n : HALF * (n + 1)], ps[:], AF.Copy
                            )
                return kt

            qt_cur = q_proj(0)
            kt_cur = kt_mm(0)
            v_cur = None

            for h in range(H):
                # prefetch next head's / pair's weights on the scalar ring
                if h + 1 < H:
                    wqk[h + 1] = w_pool.tile(
                        [P, 2, 2, F], mdt, tag="wqk", name=f"wqk{h + 1}"
                    )
                    nc.scalar.dma_start(wqk[h + 1][:], wqk_d[h + 1])
                if h % 2 == 0 and h + 2 < H:
                    hp = (h + 2) // 2
                    wvz[hp] = w_pool.tile(
                        [P, 2, 1024], mdt, tag="wvz", name=f"wvz{hp}"
                    )
                    # scalar-ring FIFO keeps this bulk load behind the
                    # critical startup transfers (SWDGE would race them)
                    nc.scalar.dma_start(wvz[hp][:], wvz_d[hp])

                # attnT_h (j 8x128, i 512) = sigmoid(16 * KT_slice^T @ QT)
                atn = attn_pool.tile([P, 8, HALF], mdt, tag="atn")
                for jb in range(8):
                    ps = ps_pool.tile([P, HALF], f32, tag="ps")
                    for c in range(2):
                        nc.tensor.matmul(
                            ps[:],
                            kt_cur[:, c, P * jb : P * (jb + 1)],
                            qt_cur[:, c, :],
                            start=(c == 0),
                            stop=(c == 1),
                        )
                    nc.scalar.activation(atn[:, jb, :], ps[:], AF.Sigmoid, scale=16.0)

                # software-pipeline: next head's Q projection keeps the PE fed
                # while the sigmoid chain drains.
                if h + 1 < H:
                    qt_next = q_proj(h + 1)

                # V for the head PAIR (j 8x128, f 2x256) = kvin^T @ [Wv_h|Wv_h+1]
                if h % 2 == 0:
                    v_cur = qkv_pool.tile([P, 8, 2 * F], mdt, tag="v")
                    for jb in range(8):
                        ps = ps_pool.tile([P, HALF], f32, tag="ps")
                        for c in range(2):
                            nc.tensor.matmul(
                                ps[:],
                                kvin[c][:, P * jb : P * (jb + 1)],
                                wvz[h // 2][:, c, 0:512],
                                start=(c == 0),
                                stop=(c == 1),
                            )
                        nc.vector.tensor_copy(v_cur[:, jb, :], ps[:])

                voff = (h % 2) * F  # this head's columns inside v_cur

                def wz_sl(c, t):
                    base = 512 + 256 * (h % 2)
                    return wvz[h // 2][:, c, base + P * t : base + P * (t + 1)]

                # wvT_h (f 2x128, i 512): c0/c1 accumulation chains
                # interleaved so the chain ends land well after the last
                # sigmoid and neither chain stalls.
                wvt = qkv_pool.tile([P, 2, HALF], mdt, tag="wvt")
                psc = [
                    ps_pool.tile([P, HALF], f32, tag="ps", name=f"pswv{h}{c}")
                    for c in range(2)
                ]
                for jb in range(8):
                    for c in range(2):
                        nc.tensor.matmul(
                            psc[c][:],
                            v_cur[:, jb, voff + P * c : voff + P * (c + 1)],
                            atn[:, jb, :],
                            start=(jb == 0),
                            stop=(jb == 7),
                        )
                nc.vector.tensor_copy(wvt[:, 0, :], psc[0][:])
                nc.scalar.activation(wvt[:, 1, :], psc[1][:], AF.Copy)

                if h < H - 1:
                    # next head's K projection pads the PE stream across the
                    # wvt-copy latency before the output projection.
                    kt_next = kt_mm(h + 1)

                    # outT (fo 2x128, i 512) += Wz_h[c]^T @ wvT[c]
                    for c in range(2):
                        for t in range(2):
                            nc.tensor.matmul(
                                out_ps[:, t, :],
                                wz_sl(c, t),
                                wvt[:, c, :],
                                start=(h == 0 and c == 0),
                                stop=False,
                            )
                    kt_cur = kt_next
                    qt_cur = qt_next
                else:
                    # last head: c-major projection order completes PSUM bank
                    # t as early as possible; relu+DMA per t on both engine
                    # pairs / rings so the drain overlaps.
                    out_sb = io_pool.tile([P, 2, HALF], mdt, tag="out_sb")
                    for c in range(2):
                        for t in range(2):
                            nc.tensor.matmul(
                                out_ps[:, t, :],
                                wz_sl(c, t),
                                wvt[:, c, :],
                                start=False,
                                stop=(c == 1),
                            )
                    nc.vector.tensor_relu(out_sb[:, 0, :], out_ps[:, 0, :])
                    nc.sync.dma_start(out_d[:, 0], out_sb[:, 0, :])
                    nc.scalar.activation(out_sb[:, 1, :], out_ps[:, 1, :], AF.Relu)
                    nc.scalar.dma_start(out_d[:, 1], out_sb[:, 1, :])

    nc.compile()
    return nc


def _get_nc():
    if "nc" not in _cache:
        _cache["nc"] = _build()
    return _cache["nc"]


def _make_in_maps(inputs):
    ndt = np.float16
    q_input = np.asarray(inputs["q_input"], dtype=np.float32)
    kv_input = np.asarray(inputs["kv_input"], dtype=np.float32)

    # Wq/Wk [f_in, fo*H] (col fi*H+hi) -> [h, p, c, fo]
    def cols_by_head(W):
        return np.asarray(W, dtype=np.float32).reshape(2, P, F, H).transpose(3, 1, 0, 2)

    WQ = cols_by_head(inputs["Wq"])  # [H, P, 2, F]
    WK = cols_by_head(inputs["Wk"])
    WQK = np.ascontiguousarray(
        np.stack([WQ, WK], axis=2), dtype=ndt
    )  # [H, P, 2(q|k), 2c, F]
    # Wv -> pair layout [4, P, 2c, 512]
    WV = cols_by_head(inputs["Wv"])  # [H, P, 2, F]
    WVP = (
        WV.reshape(4, 2, P, 2, F).transpose(0, 2, 3, 1, 4).reshape(4, P, 2, 2 * F)
    )
    # Wz [f*H, fo] (row fi*H+hi) -> [h, p, c, fo]
    WZ = (
        np.asarray(inputs["Wz"], dtype=np.float32)
        .reshape(2, P, H, F)
        .transpose(2, 1, 0, 3)
    )  # [H, P, 2, F]
    WZP = WZ.reshape(4, 2, P, 2, F).transpose(0, 2, 3, 1, 4).reshape(4, P, 2, 2 * F)
    WVZ = np.ascontiguousarray(
        np.concatenate([WVP, WZP], axis=3), dtype=ndt
    )  # [4, P, 2, 1024]

    in_maps = []
    for core in range(NCORES):
        b, half = divmod(core, 2)
        qb = q_input[b].reshape(2, P, S)
        qin = np.ascontiguousarray(
            qb[:, :, half * HALF : (half + 1) * HALF].transpose(1, 0, 2), dtype=ndt
        )
        kvin = np.ascontiguousarray(
            kv_input[b].reshape(2, P, S).transpose(1, 0, 2), dtype=ndt
        )
        in_maps.append({"qin": qin, "kvin": kvin, "wqk": WQK, "wvz": WVZ})
    return in_maps


def kernel(q_input, kv_input, Wq, Wk, Wv, Wz, **kw):
    from concourse.bass_utils import run_bass_kernel_spmd

    nc = _get_nc()
    in_maps = _make_in_maps(
        {
            "q_input": q_input,
            "kv_input": kv_input,
            "Wq": Wq,
            "Wk": Wk,
            "Wv": Wv,
            "Wz": Wz,
        }
    )

    res = run_bass_kernel_spmd(nc, in_maps, core_ids=list(range(NCORES)))

    out = np.empty((B, F, S), dtype=np.float32)
    for core in range(NCORES):
        b, half = divmod(core, 2)
        o = np.asarray(res.results[core]["out"], dtype=np.float32)  # (P, 2, HALF)
        out[b, :, half * HALF : (half + 1) * HALF] = o.transpose(1, 0, 2).reshape(
            F, HALF
        )
    return out


# revision 17
# speedup vs baseline: 1.0229x; 1.0096x over previous
"""Trainium2 Bass kernel for sigmoid-gated multi-head attention.

Reference computation (B=4, F=256, H=8, S=1024):
    qx  = q_input^T          (b, s, f)
    q   = qx @ Wq  -> (b, s, f, h)   [col fi*H + hi]
    k,v = kvx @ Wk / Wv
    attn = sigmoid(sqrt(F) * q.k)    per head
    wv   = attn @ v
    out  = relu(concat_heads(wv) @ Wz)   returned as (b, f, s)

Sharding: 8 cores = 4 batches x 2 query-sequence halves. Each core
computes the full pipeline (all 8 heads) for its (batch, s-half) slice,
including the final ReLU, so per-core outputs are disjoint slices of the
final output and no cross-core reduction is needed.

All on-chip compute keeps the "transposed" layout (feature, sequence):
    QT_h (f, i)  = Wq_h^T @ q_in       KT_h (f, j) = Wk_h^T @ kv_in
    V_hp (j, f2) = kv_in^T @ [Wv_h|Wv_h+1]   (head PAIR -> N=512 matmuls)
    attnT_h (j, i) = sigmoid(16 * KT_h^T_slice . QT_h)
    wvT_h (f, i) = V_slice^T @ attnT_h
    outT (fo, i) += Wz_h^T @ wvT_h     -> relu -> output slice

Startup: the critical first loads are spread across three DMA queues so
compute can start ~3us earlier than a two-queue layout:
    sync ring:   wqk[h0] (wq+wk), qin chunk 0, qin chunk 1
    gpsimd ring: kvin chunk 0, kvin chunk 1 (SWDGE)
    scalar ring: wvz[pair0] + all later per-head weights (the scalar
                 queue starts ~2us late behind the ACT table load)
A short warmup matmul burst bridges the DMA window and releases the PE
HAM clock gate before real matmuls arrive.

Per-head stage order:  attn -> qproj(h+1) -> V-pair (even h) -> wv
(c0/c1 interleaved) -> kt(h+1) -> zproj(h).  This keeps PE work between
every producer/consumer pair (sigmoid chain, PSUM->SBUF copies) so the
matmul stream never waits.
"""

import os
import sys

sys.path.insert(0, "/opt/trn_rl_repo")

import numpy as np

B, F, H, S = 4, 256, 8, 1024
HALF = S // 2  # query columns per core
NCORES = 8
P = 128  # partitions

_cache = {}


def _build():
    import concourse.mybir as mybir
    import concourse.tile as tile
    from concourse import bacc

    dt = mybir.dt
    f32 = dt.float32
    mdt = dt.float16
    AF = mybir.ActivationFunctionType

    nc = bacc.Bacc(None, target_bir_lowering=False)

    # all partition-major: [P, ...] with per-partition lines contiguous
    qin_d = nc.dram_tensor("qin", [P, 2, HALF], mdt, kind="ExternalInput")
    kvin_d = nc.dram_tensor("kvin", [P, 2, S], mdt, kind="ExternalInput")
    # per head: [wq|wk][f_in chunk][f_out]
    wqk_d = nc.dram_tensor("wqk", [H, P, 2, 2, F], mdt, kind="ExternalInput")
    # per head-pair: cols 0:512 = [Wv_h|Wv_h+1], 512:768 = Wz_h, 768:1024 = Wz_h+1
    wvz_d = nc.dram_tensor("wvz", [H // 2, P, 2, 1024], mdt, kind="ExternalInput")
    out_d = nc.dram_tensor("out", [P, 2, HALF], mdt, kind="ExternalOutput")

    with tile.TileContext(nc) as tc:
        with (
            tc.tile_pool(name="io", bufs=1) as io_pool,
            tc.tile_pool(name="wts", bufs=2) as w_pool,
            tc.tile_pool(name="qkv", bufs=2) as qkv_pool,
            tc.tile_pool(name="attn", bufs=2) as attn_pool,
            tc.tile_pool(name="ps", bufs=6, space="PSUM") as ps_pool,
            tc.tile_pool(name="ops", bufs=1, space="PSUM") as out_ps_pool,
        ):
            # ---- input DMAs: critical loads first, HWDGE rings only, and as
            # FEW dma_start instructions as possible (each occupies the
            # issuing sequencer ~0.65us for descriptor generation).
            # sync ring (live at ~+0.6us): wqk0, qin, kvin chunk 1.
            # scalar ring (live at ~+2.7us behind the ACT table load):
            # kvin chunk 0, wvz pair 0.
            wqk = {0: w_pool.tile([P, 2, 2, F], mdt, tag="wqk", name="wqk0")}
            nc.sync.dma_start(wqk[0][:], wqk_d[0])
            qin = io_pool.tile([P, 2, HALF], mdt, tag="qin")
            nc.sync.dma_start(qin[:], qin_d[:])
            kvin = [
                io_pool.tile([P, S], mdt, tag=f"kvin{c}", name=f"kvin{c}")
                for c in range(2)
            ]
            nc.scalar.dma_start(kvin[0][:], kvin_d[:, 0])
            nc.sync.dma_start(kvin[1][:], kvin_d[:, 1])
            wvz = {0: w_pool.tile([P, 2, 1024], mdt, tag="wvz", name="wvz0")}
            nc.scalar.dma_start(wvz[0][:], wvz_d[0])

            # ---- PE pre-warm: dummy matmuls bridge the input-DMA window and
            # release the HAM clock gate before real matmuls arrive.
            nwarm = int(os.environ.get("ATTN_NWARM", "7"))
            if nwarm:
                warm = io_pool.tile([P, HALF], dt.bfloat16, tag="warm")
                nc.gpsimd.memset(warm[:], 0.0)
                # dummy sigmoid first so the boot-time ACT table load picks
                # the sigmoid set (which also holds Copy/Relu) -- avoids a
                # second 1.3us table load in the middle of head 0's chain.
                warm16 = io_pool.tile([P, 8], mdt, tag="warm16")
                nc.scalar.activation(warm16[:], warm[:, :8], AF.Sigmoid, scale=16.0)
                wps = [
                    ps_pool.tile([P, HALF], f32, tag="ps", name=f"wps{i}")
                    for i in range(2)
                ]
                for i in range(nwarm):
                    nc.tensor.matmul(
                        wps[i % 2][:], warm[:, :P], warm[:], start=True, stop=True
                    )

            # persistent accumulator for the output projection: 2 banks
            out_ps = out_ps_pool.tile([P, 2, HALF], f32, tag="out_ps")

            def q_proj(h):
                """QT_h (f 2x128, i 512) = Wq_h^T @ qin; returns qt tile."""
                qt = qkv_pool.tile([P, 2, HALF], mdt, tag="qt", name=f"qt{h}")
                for t in range(2):
                    ps = ps_pool.tile([P, HALF], f32, tag="ps", name=f"psq{h}{t}")
                    for c in range(2):
                        nc.tensor.matmul(
                            ps[:],
                            wqk[h][:, 0, c, P * t : P * (t + 1)],
                            qin[:, c, :],
                            start=(c == 0),
                            stop=(c == 1),
                        )
                    if t == 0:
                        nc.vector.tensor_copy(qt[:, t, :], ps[:])
                    else:
                        nc.scalar.activation(qt[:, t, :], ps[:], AF.Copy)
                return qt

            def kt_mm(h):
                """KT_h (f 2x128, j 1024) = Wk_h^T @ kvin; returns kt tile."""
                kt = qkv_pool.tile([P, 2, S], mdt, tag="kt", name=f"kt{h}")
                for t in range(2):
                    for n in range(2):
                        ps = ps_pool.tile([P, HALF], f32, tag="ps")
                        for c in range(2):
                            nc.tensor.matmul(
                                ps[:],
                                wqk[h][:, 1, c, P * t : P * (t + 1)],
                                kvin[c][:, HALF * n : HALF * (n + 1)],
                                start=(c == 0),
                                stop=(c == 1),
                            )
                        if (t + n) % 2 == 0:
                            nc.vector.tensor_copy(
                                kt[:, t, HALF * n : HALF * (n + 1)], ps[:]
                            )
                        else:
                            nc.scalar.activation(
                                kt[:, t, HALF * n : HALF * (n + 1)], ps[:], AF.Copy
                            )
                return kt

            qt_cur = q_proj(0)
            kt_cur = kt_mm(0)
            v_cur = None

            for h in range(H):
                # prefetch next head's / pair's weights on the sync ring --
                # the sync sequencer is idle mid-kernel, so the ~0.65us
                # descriptor-generation time never blocks sigmoids/copies.
                if h + 1 < H:
                    wqk[h + 1] = w_pool.tile(
                        [P, 2, 2, F], mdt, tag="wqk", name=f"wqk{h + 1}"
                    )
                    nc.sync.dma_start(wqk[h + 1][:], wqk_d[h + 1])
                if h % 2 == 0 and h + 2 < H:
                    hp = (h + 2) // 2
                    wvz[hp] = w_pool.tile(
                        [P, 2, 1024], mdt, tag="wvz", name=f"wvz{hp}"
                    )
                    nc.sync.dma_start(wvz[hp][:], wvz_d[hp])

                # attnT_h (j 8x128, i 512) = sigmoid(16 * KT_slice^T @ QT)
                atn = attn_pool.tile([P, 8, HALF], mdt, tag="atn")
                for jb in range(8):
                    ps = ps_pool.tile([P, HALF], f32, tag="ps")
                    for c in range(2):
                        nc.tensor.matmul(
                            ps[:],
                            kt_cur[:, c, P * jb : P * (jb + 1)],
                            qt_cur[:, c, :],
                            start=(c == 0),
                            stop=(c == 1),
                        )
                    nc.scalar.activation(atn[:, jb, :], ps[:], AF.Sigmoid, scale=16.0)

                # software-pipeline: next head's Q projection keeps the PE fed
                # while the sigmoid chain drains.
                if h + 1 < H:
                    qt_next = q_proj(h + 1)

                # V for the head PAIR (j 8x128, f 2x256) = kvin^T @ [Wv_h|Wv_h+1]
                if h % 2 == 0:
                    v_cur = qkv_pool.tile([P, 8, 2 * F], mdt, tag="v")
                    for jb in range(8):
                        ps = ps_pool.tile([P, HALF], f32, tag="ps")
                        for c in range(2):
                            nc.tensor.matmul(
                                ps[:],
                                kvin[c][:, P * jb : P * (jb + 1)],
                                wvz[h // 2][:, c, 0:512],
                                start=(c == 0),
                                stop=(c == 1),
                            )
                        nc.vector.tensor_copy(v_cur[:, jb, :], ps[:])

                voff = (h % 2) * F  # this head's columns inside v_cur

                def wz_sl(c, t):
                    base = 512 + 256 * (h % 2)
                    return wvz[h // 2][:, c, base + P * t : base + P * (t + 1)]

                # wvT_h (f 2x128, i 512): c0/c1 accumulation chains
                # interleaved so the chain ends land well after the last
                # sigmoid and neither chain stalls.
                wvt = qkv_pool.tile([P, 2, HALF], mdt, tag="wvt")
                psc = [
                    ps_pool.tile([P, HALF], f32, tag="ps", name=f"pswv{h}{c}")
                    for c in range(2)
                ]
                for jb in range(8):
                    for c in range(2):
                        nc.tensor.matmul(
                            psc[c][:],
                            v_cur[:, jb, voff + P * c : voff + P * (c + 1)],
                            atn[:, jb, :],
                            start=(jb == 0),
                            stop=(jb == 7),
                        )
                if h == H - 1:
                    # scalar is still draining sigmoids here; vector (idle)
                    # takes the c1 copy so the final projection isn't gated
                    # on the scalar queue.
                    nc.vector.tensor_copy(wvt[:, 1, :], psc[1][:])
                    nc.scalar.activation(wvt[:, 0, :], psc[0][:], AF.Copy)
                else:
                    nc.vector.tensor_copy(wvt[:, 0, :], psc[0][:])
                    nc.scalar.activation(wvt[:, 1, :], psc[1][:], AF.Copy)

                if h < H - 1:
                    # next head's K projection pads the PE stream across the
                    # wvt-copy latency before the output projection.
                    kt_next = kt_mm(h + 1)

                    # outT (fo 2x128, i 512) += Wz_h[c]^T @ wvT[c]
                    for c in range(2):
                        for t in range(2):
                            nc.tensor.matmul(
                                out_ps[:, t, :],
                                wz_sl(c, t),
                                wvt[:, c, :],
                                start=(h == 0 and c == 0),
                                stop=False,
                            )
                    kt_cur = kt_next
                    qt_cur = qt_next
                else:
                    # last head: project c1 first (its copy lands first on the
                    # idle vector engine), then c0; relu+DMA per t on both
                    # engine pairs / rings so the drain overlaps.
                    out_sb = io_pool.tile([P, 2, HALF], mdt, tag="out_sb")
                    for c in (1, 0):
                        for t in range(2):
                            nc.tensor.matmul(
                                out_ps[:, t, :],
                                wz_sl(c, t),
                                wvt[:, c, :],
                                start=False,
                                stop=(c == 0),
                            )
                    nc.vector.tensor_relu(out_sb[:, 0, :], out_ps[:, 0, :])
                    nc.sync.dma_start(out_d[:, 0], out_sb[:, 0, :])
                    nc.scalar.activation(out_sb[:, 1, :], out_ps[:, 1, :], AF.Relu)
                    nc.scalar.dma_start(out_d[:, 1], out_sb[:, 1, :])

    nc.compile()
    return nc


def _get_nc():
    if "nc" not in _cache:
        _cache["nc"] = _build()
    return _cache["nc"]


def _make_in_maps(inputs):
    ndt = np.float16
    q_input = np.asarray(inputs["q_input"], dtype=np.float32)
    kv_input = np.asarray(inputs["kv_input"], dtype=np.float32)

    # Wq/Wk [f_in, fo*H] (col fi*H+hi) -> [h, p, c, fo]
    def cols_by_head(W):
        return np.asarray(W, dtype=np.float32).reshape(2, P, F, H).transpose(3, 1, 0, 2)

    WQ = cols_by_head(inputs["Wq"])  # [H, P, 2, F]
    WK = cols_by_head(inputs["Wk"])
    WQK = np.ascontiguousarray(
        np.stack([WQ, WK], axis=2), dtype=ndt
    )  # [H, P, 2(q|k), 2c, F]
    # Wv -> pair layout [4, P, 2c, 512]
    WV = cols_by_head(inputs["Wv"])  # [H, P, 2, F]
    WVP = (
        WV.reshape(4, 2, P, 2, F).transpose(0, 2, 3, 1, 4).reshape(4, P, 2, 2 * F)
    )
    # Wz [f*H, fo] (row fi*H+hi) -> [h, p, c, fo]
    WZ = (
        np.asarray(inputs["Wz"], dtype=np.float32)
        .reshape(2, P, H, F)
        .transpose(2, 1, 0, 3)
    )  # [H, P, 2, F]
    WZP = WZ.reshape(4, 2, P, 2, F).transpose(0, 2, 3, 1, 4).reshape(4, P, 2, 2 * F)
    WVZ = np.ascontiguousarray(
        np.concatenate([WVP, WZP], axis=3), dtype=ndt
    )  # [4, P, 2, 1024]

    in_maps = []
    for core in range(NCORES):
        b, half = divmod(core, 2)
        qb = q_input[b].reshape(2, P, S)
        qin = np.ascontiguousarray(
            qb[:, :, half * HALF : (half + 1) * HALF].transpose(1, 0, 2), dtype=ndt
        )
        kvin = np.ascontiguousarray(
            kv_input[b].reshape(2, P, S).transpose(1, 0, 2), dtype=ndt
        )
        in_maps.append({"qin": qin, "kvin": kvin, "wqk": WQK, "wvz": WVZ})
    return in_maps


def kernel(q_input, kv_input, Wq, Wk, Wv, Wz, **kw):
    from concourse.bass_utils import run_bass_kernel_spmd

    nc = _get_nc()
    in_maps = _make_in_maps(
        {
            "q_input": q_input,
            "kv_input": kv_input,
            "Wq": Wq,
            "Wk": Wk,
            "Wv": Wv,
            "Wz": Wz,
        }
    )

    res = run_bass_kernel_spmd(nc, in_maps, core_ids=list(range(NCORES)))

    out = np.empty((B, F, S), dtype=np.float32)
    for core in range(NCORES):
        b, half = divmod(core, 2)
        o = np.asarray(res.results[core]["out"], dtype=np.float32)  # (P, 2, HALF)
        out[b, :, half * HALF : (half + 1) * HALF] = o.transpose(1, 0, 2).reshape(
            F, HALF
        )
    return out
